# revision 1
# baseline (speedup 1.0000x reference)
"""Trainium2 Bass kernel for nn_MinEuclideanDistBlock.

Math (reference):
  x: (B=64, C=3, L=2048), shapelets: (C=3, N=256, S=64)
  W = L - S + 1 = 1985 sliding windows
  d2[b,c,w,n] = |win|^2 + |shp|^2 - 2 win.shp    (win = x[b,c,w:w+S])
  d = sqrt(max(d2, 0));  out[b,0,n] = min_w sum_c d[b,c,w,n]

Device strategy (per core, batch-sharded B/8 = 8 batches per core):
  - win_sq rows are precomputed on host (pure function of x) and shipped
    as the `wsq` input, removing the on-device prefix-sum prep.
  - T matrices (T[s,w] = x[b,c,w+s], row 64 = win_sq) for all 24 (b,c)
    are loaded ONCE per NEFF into persistent SBUF tiles, in groups of 8
    via a single 3D-access-pattern SWDGE DMA each (48 small HWDGE DMAs
    per pass measured ~150us -- per-DMA completion dominates; batching
    + hoisting takes DMA off the steady-state critical path entirely).
  - lhsT[k, n] (host precomputed): rows 0..63 = -2*shapelets^T, row 64 = 1.
    One bf16 matmul per 512-wide w chunk gives psum[n,w] = -2*cross + win_sq.
  - ACT: d = sqrt(psum + bias) with per-partition bias = shp_sq[n],
    output bf16 (enables the DVE 2x mode downstream). ACT measured at
    ~(FD/2+352)/1.2 ns per op => ~54us/core floor for the 48 sqrt batches.
  - DVE: channel-sum adds in bf16 (2x); 30% of the first add offloaded
    to GPSIMD. Min-over-w via tensor_scalar with accum_out (min) -- the
    dedicated tensor_tensor_reduce instruction crashes the runtime and
    tensor_reduce is capped at 1x.
  - All 16 per-(b,nt) results accumulate into one [128, 16] SBUF tile,
    written out with a single DMA.
"""

import numpy as np

S = 64
NSH = 256
C = 3
B = 64
L = 2048
W = L - S + 1  # 1985
NCORES = 8
BPC = B // NCORES  # 8
NT = 2  # shapelet tiles of 128
WCHUNKS = [(0, 512), (512, 512), (1024, 512), (1536, W - 1536)]

# tunable engine splits
C1_GPS_FRAC = 30  # percent of c1 add width on gpsimd (0 = all DVE)
C2_GPS_FRAC = 30  # percent of c2 add width on gpsimd
# T-build DMA issue mode: "batched_sw" = one 8-bc 3D DMA via SWDGE (gpsimd),
# "percbc_2ring" = per-bc 2D DMAs alternating sync/scalar HWDGE rings,
# "percbc" = per-bc 2D DMAs all on sync (baseline-style)
DMA_MODE = "batched_sw"
LDW_DEDUP = True
LOOP_MODE = "cjb"  # "bjc" = b-major (c,nt inner); "cjb" = c,nt-major within group

_cache = {}


def _build_nc(reps=1, ablate=()):
    import concourse.bass as bass
    import concourse.bacc as bacc
    import concourse.mybir as mybir
    import concourse.tile as tile

    f32 = mybir.dt.float32
    bf16 = mybir.dt.bfloat16

    nc = bacc.Bacc()
    xs = nc.dram_tensor("xs", [BPC, C, L], bf16, kind="ExternalInput")
    wts = nc.dram_tensor("wts", [C, NT, S + 1, 128], bf16, kind="ExternalInput")
    ssq = nc.dram_tensor("ssq", [C, NT, 128], f32, kind="ExternalInput")
    wsq = nc.dram_tensor("wsq", [BPC * C, L], bf16, kind="ExternalInput")
    out = nc.dram_tensor("out", [128, BPC * NT], f32, kind="ExternalOutput")

    with tile.TileContext(nc) as tc:
        with (
            tc.tile_pool(name="consts", bufs=1) as consts,
            tc.tile_pool(name="prep", bufs=1) as prep,
            tc.tile_pool(name="tpool", bufs=1) as tpool,
            tc.tile_pool(name="psump", bufs=2, space="PSUM") as psump,
            tc.tile_pool(name="accp", bufs=4) as accp,
            tc.tile_pool(name="tmpp", bufs=4) as tmpp,
            tc.tile_pool(name="minvp", bufs=8) as minvp,
        ):
            # ---- constants ----
            w_all = consts.tile([S + 1, C * NT * 128], bf16)
            biases = {}
            for c in range(C):
                for nt in range(NT):
                    idx = c * NT + nt
                    nc.sync.dma_start(
                        out=w_all[:, idx * 128 : (idx + 1) * 128],
                        in_=wts[c, nt, :, :],
                    )
                    bt = consts.tile([128, 1], f32, name=f"bias_{c}_{nt}")
                    nc.sync.dma_start(out=bt, in_=ssq[c, nt, :])
                    biases[(c, nt)] = bt

            # ---- hoisted T loads: content is invariant across reps ----
            GB = 12
            NG = BPC * C // GB  # 2
            talls = []
            for g in range(NG):
                Tall = tpool.tile([S + 1, GB, L], bf16, name=f"Tall{g}")
                bc0 = g * GB
                if "nodma" not in ablate:
                    base = xs[bc0 // C, bc0 % C, :]
                    apov = bass.AP(
                        tensor=base.tensor,
                        offset=base.offset,
                        ap=[[1, S], [L, GB], [1, W]],
                    )
                    nc.gpsimd.dma_start(out=Tall[0:S, 0:GB, 0:W], in_=apov)
                    nc.gpsimd.dma_start(
                        out=Tall[S : S + 1, 0:GB, 0:W],
                        in_=wsq[bc0 : bc0 + GB, 0:W],
                    )
                talls.append(Tall)
            for _rep in range(reps):
                _body(nc, tc, bass, mybir, talls, GB, psump, accp, tmpp,
                      minvp, xs, wsq, out, w_all, biases, ablate)
    if LDW_DEDUP:
        _dedup_ldweights(nc)
    nc.compile()
    return nc


def _dedup_ldweights(nc):
    """Drop Ldweights that reload the exact weights already resident in the
    PE array (same source AP as the previous Ldweights, nothing between
    them that could clobber the array). Tile emits one Ldweights per
    matmul even when 4 chunk-matmuls share a weight tile; the redundant
    reloads serialize the PE (~107ns each, unoverlapped). Only duplicates
    with no semaphore waits/updates are removed."""
    removed = 0
    for blk in nc.m.functions[0].blocks:
        prev_sig = None
        keep = []
        for inst in blk.instructions:
            if inst.opcode == "Ldweights":
                sig = str(inst.ins[0])
                si = inst.sync_info
                clean = si is None or (
                    len(si.on_wait) == 0 and len(si.on_update) == 0
                )
                if sig == prev_sig and clean:
                    removed += 1
                    continue
                prev_sig = sig
            keep.append(inst)
        if removed:
            blk.instructions = keep
    return removed


def _body(nc, tc, bass, mybir, talls, GB, psump, accp, tmpp, minvp,
          xs, wsq, out, w_all, biases, ablate=()):
    f32 = mybir.dt.float32
    bf16 = mybir.dt.bfloat16
    AF = mybir.ActivationFunctionType
    ALU = mybir.AluOpType

    mm_on = "nomm" not in ablate
    act_on = mm_on and "noact" not in ablate
    add_on = act_on and "noadd" not in ablate
    red_on = add_on and "nored" not in ablate

    accs = {}
    minv_all = minvp.tile([128, BPC * NT], f32, name="minv_all")
    for g in range(len(talls)):
        Tall = talls[g]
        bc0 = g * GB
        if LOOP_MODE == "cjb":
            # (c, nt)-major: consecutive matmul sets share lhsT, so the
            # Ldweights dedup collapses them to one weight load per group
            order = [
                (j, nt)
                for c in range(C)
                for nt in range(NT)
                for j in range(GB)
                if (bc0 + j) % C == c
            ]
        else:
            order = [(j, nt) for j in range(GB) for nt in range(NT)]
        for j, nt in order:
            bc = bc0 + j
            b, c = bc // C, bc % C
            if True:
                idx = c * NT + nt
                lhsT = w_all[:, idx * 128 : (idx + 1) * 128]
                if mm_on:
                    ps = psump.tile([128, 2048], f32, name="ps")
                    for w0, wl in WCHUNKS:
                        nc.tensor.matmul(
                            ps[:, w0 : w0 + wl],
                            lhsT=lhsT,
                            rhs=Tall[:, j, w0 : w0 + wl],
                            start=True,
                            stop=True,
                        )
                if c == 0:
                    if act_on:
                        acc = accp.tile([128, W], bf16, name=f"acc{nt}")
                        accs[(b, nt)] = acc
                        nc.scalar.activation(
                            acc[:, 0:W], ps[:, 0:W], AF.Sqrt, bias=biases[(c, nt)]
                        )
                elif c == 1:
                    if act_on:
                        tmp = tmpp.tile([128, W], bf16, name="tmp")
                        nc.scalar.activation(
                            tmp[:, 0:W], ps[:, 0:W], AF.Sqrt, bias=biases[(c, nt)]
                        )
                    if add_on:
                        A1 = (W * C1_GPS_FRAC // 100) & ~1
                        if A1 > 0:
                            nc.gpsimd.tensor_add(
                                accs[(b, nt)][:, 0:A1], accs[(b, nt)][:, 0:A1], tmp[:, 0:A1]
                            )
                        nc.vector.tensor_add(
                            accs[(b, nt)][:, A1:W], accs[(b, nt)][:, A1:W], tmp[:, A1:W]
                        )
                else:
                    if act_on:
                        tmp = tmpp.tile([128, W], bf16, name="tmp")
                        nc.scalar.activation(
                            tmp[:, 0:W], ps[:, 0:W], AF.Sqrt, bias=biases[(c, nt)]
                        )
                    col = b * NT + nt
                    if add_on:
                        scratch = tmpp.tile([128, W], bf16, name="scratch")
                        A2 = (W * C2_GPS_FRAC // 100) & ~1
                        if A2 > 0:
                            nc.gpsimd.tensor_add(
                                scratch[:, 0:A2], accs[(b, nt)][:, 0:A2],
                                tmp[:, 0:A2]
                            )
                        nc.vector.tensor_add(
                            scratch[:, A2:W], accs[(b, nt)][:, A2:W],
                            tmp[:, A2:W]
                        )
                    if red_on:
                        # min-reduce via tensor_scalar accumulate (2x bf16)
                        trash = tmpp.tile([128, W], bf16, name="trash")
                        nc.vector.tensor_scalar(
                            out=trash, in0=scratch[:, 0:W], scalar1=0.0,
                            scalar2=None, op0=ALU.add, op1=ALU.min,
                            accum_out=minv_all[:, col : col + 1],
                        )
                    else:
                        nc.vector.tensor_copy(
                            minv_all[:, col : col + 1], biases[(0, nt)]
                        )
    nc.sync.dma_start(out=out[:, :], in_=minv_all)


def _get_nc():
    if "nc" not in _cache:
        _cache["nc"] = _build_nc()
    return _cache["nc"]


def _prep_inputs(x, shapelets):
    import ml_dtypes

    bf16 = ml_dtypes.bfloat16
    x = np.ascontiguousarray(np.asarray(x), dtype=np.float32)
    sh = np.asarray(shapelets, dtype=np.float32)
    # round shapelets to bf16 once; all derived quantities use the rounded
    # values so d2 stays an exact squared distance of the rounded vectors
    shb = sh.astype(bf16).astype(np.float32)
    shT = np.transpose(shb, (0, 2, 1))  # (C, S, N)
    wts = np.empty((C, NT, S + 1, 128), np.float32)
    for nt in range(NT):
        wts[:, nt, :S, :] = -2.0 * shT[:, :, nt * 128 : (nt + 1) * 128]
    wts[:, :, S, :] = 1.0
    ssq = np.sum(shb * shb, axis=2).reshape(C, NT, 128).astype(np.float32)
    wts_b = np.ascontiguousarray(wts.astype(bf16))
    ssq = np.ascontiguousarray(ssq)
    xb = x.astype(bf16)
    # win_sq from the bf16-rounded x (what the device matmul sees)
    xf = xb.astype(np.float32)
    xsq = xf * xf
    cums = np.concatenate(
        [np.zeros((B, C, 1), np.float32), np.cumsum(xsq, axis=2)], axis=2
    )
    winsq_full = np.zeros((B, C, L), np.float32)
    winsq_full[:, :, :W] = cums[:, :, S : L + 1] - cums[:, :, 0:W]
    wsq_b = winsq_full.astype(bf16)
    in_maps = [
        {
            "xs": np.ascontiguousarray(xb[k * BPC : (k + 1) * BPC]),
            "wts": wts_b,
            "ssq": ssq,
            "wsq": np.ascontiguousarray(
                wsq_b[k * BPC : (k + 1) * BPC].reshape(BPC * C, L)
            ),
        }
        for k in range(NCORES)
    ]
    return in_maps


def _gather(results):
    outs = []
    for r in results:
        o = np.asarray(r["out"]).reshape(128, BPC, NT)  # [n128, b, nt]
        outs.append(np.transpose(o, (1, 2, 0)).reshape(BPC, NSH))
    full = np.concatenate(outs, axis=0)  # (64, 256)
    return np.ascontiguousarray(full[:, None, :]).astype(np.float32)  # (64, 1, 256)


def kernel(x, shapelets):
    from concourse.bass_utils import run_bass_kernel_spmd

    nc = _get_nc()
    in_maps = _prep_inputs(x, shapelets)
    res = run_bass_kernel_spmd(nc, in_maps, core_ids=list(range(NCORES)))
    return _gather(res.results)


def kernel_traced(x, shapelets):
    """Like kernel() but requests an NTFF trace; returns (out, BassKernelResults)."""
    from concourse.bass_utils import run_bass_kernel_spmd

    nc = _get_nc()
    in_maps = _prep_inputs(x, shapelets)
    res = run_bass_kernel_spmd(nc, in_maps, core_ids=list(range(NCORES)), trace=True)
    return _gather(res.results), res

